# revision 1
# baseline (speedup 1.0000x reference)
"""FourierKAN adapter kernel for Trainium2 (8 NeuronCores, SPMD data-parallel).

out[t, d] = x[t, d] + c0[d] + sum_{k=1..3} a_k[d] sin(k x) + b_k[d] cos(k x)
x: [32768, 1024] f32, coeffs: [1024, 7] f32.

Math: a sin(kx) + b cos(kx) = r_k sin(k x + phi_k), r = hypot(a, b),
phi = atan2(b, a). On-chip, per harmonic (fp16 "turns" representation):
    w  = x / (2 pi)                                    (fp16 master)
    t  = w + phi_k/(2 pi k)                            (phase add, fp16)
    y  = fp16(k*t + M), M = 1536                       (fp16 round-to-int trick)
    u  = t - (y - M)/k                                 (|2 pi k u| <= pi)
    s  = Sin(2 pi k * u)       (ScalarE spline, in-domain)
    m  = s * r_k
then PSUM accumulation on TensorE: psum = x + c0 + m1 + m2 + m3 (diagonal
identity matmuls; x via fp32 matmul; c0 via K=1 ones-row matmul), evacuated
with a VectorE fp32 copy.

Sharding: x row-sharded across 8 cores; coeff-derived tables replicated.
"""

import os
import numpy as np

T = 32768
D = 1024
K = 3
N_CORES = 8
T_CORE = T // N_CORES  # 4096
P = 128
F = 2048               # megatile free dim (= 2 d-periods)
REP = D // 1           # table tiling helper
M16 = 1536.0           # fp16 magic rounding constant (ulp 1 in [1024,2048))
TWO_PI = 2.0 * np.pi

LAST_RESULTS = None
_CACHED = {}


def _build_nc(reps=1, mode='nogp'):
    from concourse import bacc
    import concourse.mybir as mybir
    from concourse import tile
    from concourse.alu_op_type import AluOpType

    f32 = mybir.dt.float32
    f16 = mybir.dt.float16
    Sin = mybir.ActivationFunctionType.Sin

    nc = bacc.Bacc("TRN2", target_bir_lowering=False, debug=False)

    x = nc.dram_tensor("x", [T_CORE, D], f32, kind="ExternalInput").ap()
    out = nc.dram_tensor("out", [T_CORE, D], f32, kind="ExternalOutput").ap()

    tab = {}
    for k in (1, 2, 3):
        tab[f"phit{k}"] = nc.dram_tensor(f"phit{k}", [P, F], f16, kind="ExternalInput").ap()
        tab[f"rb{k}"] = nc.dram_tensor(f"rb{k}", [P, F], f16, kind="ExternalInput").ap()
    c0row = nc.dram_tensor("c0row", [1, F], f16, kind="ExternalInput").ap()
    id16 = nc.dram_tensor("id16", [P, P], f16, kind="ExternalInput").ap()
    id32 = nc.dram_tensor("id32", [P, P], f32, kind="ExternalInput").ap()
    ones1 = nc.dram_tensor("ones1", [1, P], f16, kind="ExternalInput").ap()

    xv = x.rearrange("(a b) d -> a (b d)", b=F // D)     # [2048, 2048]
    ov = out.rearrange("(a b) d -> a (b d)", b=F // D)
    n_tiles = xv.shape[0] // P  # 16

    with tile.TileContext(nc) as tc:
        with (
            tc.tile_pool(name="consts", bufs=1) as cpool,
            tc.tile_pool(name="io", bufs=4) as iopool,
            tc.tile_pool(name="work", bufs=4) as pool,
            tc.tile_pool(name="mwork", bufs=3) as mpool,
            tc.tile_pool(name="psum", bufs=2, space="PSUM") as ppool,
        ):
            sb = {}
            for name, ap in tab.items():
                t_ = cpool.tile([P, F], f16, tag=name)
                nc.sync.dma_start(out=t_[:], in_=ap)
                sb[name] = t_
            c0t = cpool.tile([1, F], f16, tag="c0row")
            nc.sync.dma_start(out=c0t[:], in_=c0row)
            id16t = cpool.tile([P, P], f16, tag="id16")
            nc.sync.dma_start(out=id16t[:], in_=id16)
            id32t = cpool.tile([P, P], f32, tag="id32")
            nc.sync.dma_start(out=id32t[:], in_=id32)
            ones1t = cpool.tile([1, P], f16, tag="ones1")
            nc.sync.dma_start(out=ones1t[:], in_=ones1)

            def body():
                for i in range(n_tiles):
                    xt = iopool.tile([P, F], f32, tag="xt")
                    nc.sync.dma_start(out=xt[:], in_=xv[i * P:(i + 1) * P])

                    if mode == "dma":
                        nc.sync.dma_start(out=ov[i * P:(i + 1) * P], in_=xt[:])
                        continue

                    w = pool.tile([P, F], f16, tag="w")
                    weng = nc.vector if mode == "nogp" else nc.gpsimd
                    weng.tensor_scalar(
                        out=w[:], in0=xt[:], scalar1=1.0 / TWO_PI, scalar2=None,
                        op0=AluOpType.mult)

                    ms = []
                    Copy = mybir.ActivationFunctionType.Copy
                    for k in (1, 2, 3):
                        tt = pool.tile([P, F], f16, tag="tt")
                        nc.vector.tensor_add(out=tt[:], in0=w[:], in1=sb[f"phit{k}"][:])
                        y = pool.tile([P, F], f16, tag="y")
                        if k in (1, 2):
                            # fp16(tt + M/k) rounds tt to multiples of 1/k
                            # (ulp(M/k..2M/k) = 1/k for M = 1536).
                            Mk = M16 / k
                            nc.scalar.activation(y[:], tt[:], Copy,
                                                 bias=Mk, scale=1.0)
                            # un = (y - Mk) - tt = n/k - tt  (negated angle)
                            nc.vector.scalar_tensor_tensor(
                                out=y[:], in0=y[:], scalar=Mk, in1=tt[:],
                                op0=AluOpType.subtract, op1=AluOpType.subtract)
                            nc.scalar.activation(tt[:], y[:], Sin, bias=0.0,
                                                 scale=float(-TWO_PI * k))
                        else:
                            # y = fp16(k*tt + M) rounds to integer n
                            nc.scalar.activation(y[:], tt[:], Copy,
                                                 bias=M16, scale=float(k))
                            # y <- (y - M)/k ; u(tt) <- tt - y ; s(tt) <- Sin
                            nc.vector.tensor_scalar(
                                out=y[:], in0=y[:], scalar1=M16, scalar2=1.0 / k,
                                op0=AluOpType.subtract, op1=AluOpType.mult)
                            nc.vector.tensor_sub(out=tt[:], in0=tt[:], in1=y[:])
                            nc.scalar.activation(tt[:], tt[:], Sin, bias=0.0,
                                                 scale=float(TWO_PI * k))
                        m = mpool.tile([P, F], f16, tag=f"m{k}")
                        eng = nc.gpsimd if (k == 3 and mode == "full") else nc.vector
                        eng.tensor_mul(out=m[:], in0=tt[:], in1=sb[f"rb{k}"][:])
                        ms.append(m)

                    if mode == "nope":
                        ot = iopool.tile([P, F], f32, tag="ot")
                        nc.vector.tensor_copy(out=ot[:], in_=ms[0][:])
                        nc.sync.dma_start(out=ov[i * P:(i + 1) * P], in_=ot[:])
                        continue

                    if mode == "dveonly":
                        nc.sync.dma_start(out=ov[i * P:(i + 1) * P], in_=xt[:])
                        continue

                    ps = ppool.tile([P, F], f32, tag="ps")
                    nchunk = F // 512
                    for mi, m in enumerate(ms):
                        for c in range(nchunk):
                            sl = slice(c * 512, (c + 1) * 512)
                            nc.tensor.matmul(ps[:, sl], id16t[:], m[:, sl],
                                             start=(mi == 0), stop=False)
                    for c in range(nchunk):
                        sl = slice(c * 512, (c + 1) * 512)
                        nc.tensor.matmul(ps[:, sl], ones1t[:], c0t[:, sl],
                                         start=False, stop=False)
                    for c in range(nchunk):
                        sl = slice(c * 512, (c + 1) * 512)
                        nc.tensor.matmul(ps[:, sl], id32t[:], xt[:, sl],
                                         start=False, stop=True)

                    ot = iopool.tile([P, F], f32, tag="ot")
                    nc.vector.tensor_copy(out=ot[:], in_=ps[:])
                    nc.sync.dma_start(out=ov[i * P:(i + 1) * P], in_=ot[:])

            if reps == 1:
                body()
            else:
                with tc.For_i(0, reps, 1):
                    body()

    nc.compile()
    return nc


def _host_tables(coeffs: np.ndarray) -> dict:
    c = coeffs.astype(np.float64)
    c0 = c[:, 0]
    nrep = F // D
    tabs = {"c0row": np.tile(c0, nrep)[None, :].astype(np.float16)}
    for k in (1, 2, 3):
        a = c[:, 2 * k - 1]
        b = c[:, 2 * k]
        r = np.hypot(a, b)
        phi = np.arctan2(b, a)
        tabs[f"phit{k}"] = np.tile(
            (phi / (TWO_PI * k)).astype(np.float16), (P, nrep))
        tabs[f"rb{k}"] = np.tile(r.astype(np.float16), (P, nrep))
    tabs["id16"] = np.eye(P, dtype=np.float16)
    tabs["id32"] = np.eye(P, dtype=np.float32)
    tabs["ones1"] = np.ones((1, P), dtype=np.float16)
    return tabs


def kernel(x: np.ndarray, coeffs: np.ndarray) -> np.ndarray:
    global LAST_RESULTS
    from concourse.bass_utils import run_bass_kernel_spmd

    x = np.ascontiguousarray(np.asarray(x, dtype=np.float32))
    coeffs = np.asarray(coeffs, dtype=np.float32)
    assert x.shape == (T, D) and coeffs.shape == (D, 2 * K + 1)

    if "nc" not in _CACHED:
        _CACHED["nc"] = _build_nc()
    nc = _CACHED["nc"]

    tabs = _host_tables(coeffs)
    in_maps = []
    for i in range(N_CORES):
        m = {"x": x[i * T_CORE:(i + 1) * T_CORE]}
        m.update(tabs)
        in_maps.append(m)

    res = run_bass_kernel_spmd(
        nc, in_maps, list(range(N_CORES)),
        trace=bool(os.environ.get("BASS_TRACE")),
    )
    LAST_RESULTS = res
    out = np.concatenate([res.results[i]["out"] for i in range(N_CORES)], axis=0)
    return out.astype(np.float32)

